# revision 8
# baseline (speedup 1.0000x reference)
# Transformer-XL style relative-position attention on 8 Trainium2 NeuronCores.
#
# Contract: kernel(**inputs) takes the FULL unsharded inputs and returns the
# FULL [8, 256, 1024] output. Internally shards data-parallel over batch:
# core b computes batch element b. No collectives needed.
#
# Math (per batch element):
#   cat = [h; x]                            [512, 1024]
#   q,k,v = split(cat @ Wqkv)               heads=16, dhead=64
#   RW    = R @ Wkr                         [1024, 1024] (relative pos keys)
#   dots  = (q+u) @ k^T + rel_shift((q+v) @ RW_h^T)
#   out   = softmax(dots*8^-1 + causal/mem band mask) @ v @ Wout
#
# Key design points (v2):
#  * Valid relative offsets j - i are in [0, 256]; in rel-coordinate
#    s = j - i + 256 the window is s in [256, 512] (257 values), so only 257
#    rows of RW are ever needed (R rows 768..1023 and 0).
#  * rel_shift is a per-row shear realized through a DRAM scratch: write the
#    [128, 257] valid band of BDs = (q+v) @ RWs^T to a [128, 767] buffer
#    pre-filled with the additive mask value NEG, read it back with access
#    pattern [[766, 128], [1, 384]] (row stride 767-1) which realizes
#    band[i, j] = BDs[i, j - i + const] with mask outside the band.
#  * Attention runs over HEAD PAIRS (one 128-feature tile = 2 heads):
#    - BD and A score matmuls for the two heads are row-tiled (K=64 each,
#      partitions 0:64 / 64:128) and issued back-to-back so the PE runs them
#      concurrently in different row groups.
#    - The 4 band tiles of a pair (2 heads x 2 query blocks) go to DRAM in
#      ONE write DMA and come back in ONE SWDGE read DMA with accum_op=add,
#      which adds band+mask directly onto the A scores (term_a) in SBUF --
#      no vector-engine add needed.
#    - One wide EXP activation [128, 4*384] per pair; row sums via a single
#      DVE tensor_reduce on the 3D view; normalization as 4 tensor_scalar
#      muls split across Vector/GpSimd.
#    - Normalized attn is PE-transposed (f16 PSUM) into key-major tiles; the
#      AV matmuls are column-tiled (two heads into partition halves of one
#      PSUM tile) so the pair shares one accumulation chain.
#    - The loop is software-pipelined: pair ft's scores (BD/A/DMA) are
#      emitted before pair ft-1's exp/transpose/AV so the PE never waits on
#      the DMA+exp latency chain.
#  * All matmul operands are fp16; accumulation fp32 in PSUM.
#  * Weights are cast f32->f16 in-flight by gpsimd (SWDGE) cast-DMAs, batched
#    as quad-row-block transfers (2MB apiece) to amortize Q7 dispatch cost.
#  * The Exp activation table is preloaded at t=0 so the first attention pair
#    does not pay the ~2.7us table-load.

import numpy as np

import concourse.bass as bass
import concourse.mybir as mybir
import concourse.tile as tile
from concourse import bacc, bass_utils
from concourse.masks import make_identity
from concourse.tile import add_dep_helper
from contextlib import ExitStack

F32 = mybir.dt.float32
F16 = mybir.dt.float16
AF = mybir.ActivationFunctionType
ALU_ADD = mybir.AluOpType.add
AX_X = mybir.AxisListType.X

DIM = 1024
HEADS = 16
DHEAD = 64
B = 8
N = 256          # query tokens (x)
M = 256          # memory tokens (h)
T = M + N        # 512 keys
INNER = HEADS * DHEAD
SCALE = DHEAD ** -0.5
NEG = -30000.0   # fp16-representable; *0.125 still underflows exp
SW = 767         # BDs scratch width (relative offsets s = 1..767)
VAL0 = 255       # scratch col of first valid offset (s = 256)
NVALID = 257     # valid offsets s in [256, 512]
WIN = 384        # per-query-block live key window (3 of 4 key tiles)
NGRP = 3         # scratch groups in flight (4 buffers each)
NBUF = 4 * NGRP


def build_kernel():
    nc = bacc.Bacc("TRN2", target_bir_lowering=False, debug=False)

    x_d = nc.dram_tensor("x", [N, DIM], F32, kind="ExternalInput")
    h_d = nc.dram_tensor("h", [M, DIM], F32, kind="ExternalInput")
    wqkv_d = nc.dram_tensor("Wqkv", [DIM, 3 * INNER], F32, kind="ExternalInput")
    wkr_d = nc.dram_tensor("Wkr", [DIM, INNER], F32, kind="ExternalInput")
    r_d = nc.dram_tensor("R", [2 * T, DIM], F32, kind="ExternalInput")
    uu_d = nc.dram_tensor("uu", [128, 1], F32, kind="ExternalInput")
    vv_d = nc.dram_tensor("vv", [128, 1], F32, kind="ExternalInput")
    wout_d = nc.dram_tensor("Wout", [INNER, DIM], F32, kind="ExternalInput")
    out_d = nc.dram_tensor("out", [N, DIM], F32, kind="ExternalOutput")
    bds_d = nc.dram_tensor("bds_scratch", [NBUF, 128, SW], F16)
    junk_d = nc.dram_tensor("warm_junk", [128, 512], F16)

    with tile.TileContext(nc) as tc, ExitStack() as ctx:
        _body(ctx, tc, x_d, h_d, wqkv_d, wkr_d, r_d, uu_d, vv_d, wout_d,
              out_d, bds_d, junk_d)

    nc.compile()
    return nc


def _body(ctx, tc, x_d, h_d, wqkv_d, wkr_d, r_d, uu_d, vv_d, wout_d, out_d,
          bds_d, junk_d):
    nc = tc.nc

    const = ctx.enter_context(tc.tile_pool(name="const", bufs=1))
    persist = ctx.enter_context(tc.tile_pool(name="persist", bufs=1))
    ldpool = ctx.enter_context(tc.tile_pool(name="ld", bufs=1))
    work = ctx.enter_context(tc.tile_pool(name="work", bufs=2))
    ps_mid = ctx.enter_context(tc.tile_pool(name="ps_mid", bufs=5, space="PSUM"))
    ps_sml = ctx.enter_context(tc.tile_pool(name="ps_sml", bufs=3, space="PSUM"))

    # ---------------- PE warm-up (primes the HAM clock gate) ----------------
    junk = const.tile([128, 512], F16, tag="junk", name="junk")
    nc.vector.memset(junk, 1.0)
    pwarm = ps_mid.tile([128, 512], F32, tag="mid", name="ps_warm")
    for wi in range(16):
        nc.tensor.matmul(pwarm, junk[:, 0:128], junk,
                         start=(wi == 0), stop=(wi == 15))
    junk2 = const.tile([128, 512], F16, tag="junk2", name="junk2")
    nc.vector.tensor_copy(junk2, pwarm)
    nc.sync.dma_start(out=junk_d[:, :], in_=junk2)

    # Preload the Exp activation table while DMAs stream (one tiny exp).
    pre = const.tile([128, 1], F32, tag="pre", name="pre")
    nc.gpsimd.memset(pre, 0.0)
    nc.scalar.activation(pre, pre, AF.Exp, bias=0.0, scale=1.0)

    # ---------------- constants ----------------
    ident = const.tile([128, 128], F32, tag="ident", name="ident")
    make_identity(nc, ident)
    ident_h = const.tile([128, 128], F16, tag="identh", name="ident_h")
    make_identity(nc, ident_h)

    uu = const.tile([128, 1], F32, tag="uu", name="uu_sb")
    vv = const.tile([128, 1], F32, tag="vv", name="vv_sb")
    nc.sync.dma_start(out=uu, in_=uu_d[:, :])
    nc.sync.dma_start(out=vv, in_=vv_d[:, :])

    # Scratch mask fill: all columns the shear-read can see outside the
    # per-pair band write region stay NEG forever (writes never touch them).
    maskw = const.tile([128, NBUF * 128], F16, tag="maskw", name="maskw")
    nc.vector.memset(maskw, NEG)
    zi1 = nc.sync.dma_start(
        out=bass.AP(bds_d, 127,
                    [[SW, 128], [128 * SW, NBUF], [1, 128]]),
        in_=maskw)
    zi2 = nc.sync.dma_start(
        out=bass.AP(bds_d, 512,
                    [[SW, 128], [128 * SW, NBUF], [1, 128]]),
        in_=maskw)
    zinit = (zi1, zi2)

    # ---------------- activation / R loads (gpsimd cast-DMAs) ----------------
    # cat token order: [h (0:256) | x (256:512)]
    catx = ldpool.tile([128, 2, DIM], F16, tag="catx", name="catx")
    cath = ldpool.tile([128, 2, DIM], F16, tag="cath", name="cath")
    nc.gpsimd.dma_start(
        out=cath, in_=bass.AP(h_d, 0, [[DIM, 128], [128 * DIM, 2], [1, DIM]]))
    nc.gpsimd.dma_start(
        out=catx, in_=bass.AP(x_d, 0, [[DIM, 128], [128 * DIM, 2], [1, DIM]]))

    # R rows needed: offsets s=256..511 -> rows 768..1023; s=512 -> row 0
    r16 = ldpool.tile([128, 2, DIM], F16, tag="r16", name="r16")
    nc.gpsimd.dma_start(
        out=r16, in_=bass.AP(r_d, 768 * DIM,
                             [[DIM, 128], [128 * DIM, 2], [1, DIM]]))

    # ---------------- weight loads (gpsimd cast-DMAs, quad row-blocks) -------
    # Wqkv [1024, 3072]: per projection 2 quads of 4 row-blocks x 1024 cols.
    def quad_load(dst_tag, dram_t, col0, ncols, nquads=2):
        tiles = []
        for qd in range(nquads):
            t_ = persist.tile([128, 4, ncols], F16, tag=f"{dst_tag}{qd}",
                              name=f"{dst_tag}{qd}")
            src = bass.AP(dram_t,
                          qd * 4 * 128 * (dram_t.shape[-1]) + col0,
                          [[dram_t.shape[-1], 128],
                           [128 * dram_t.shape[-1], 4],
                           [1, ncols]])
            tiles.append((t_, src))
        return tiles

    wq_t = quad_load("wq", wqkv_d, 0, INNER)
    wkr_t = quad_load("wkr", wkr_d, 0, INNER)
    wk_t = quad_load("wk", wqkv_d, INNER, INNER)
    wv_t = quad_load("wv", wqkv_d, 2 * INNER, INNER)
    wo_t = quad_load("wo", wout_d, 0, DIM)
    for t_, src in wq_t + wkr_t:
        nc.gpsimd.dma_start(out=t_, in_=src)

    def wsl(tiles, dt):
        return tiles[dt // 4][0][:, dt % 4]

    # ---------------- transpose x, h, R ----------------
    cat16 = [cath[:, 0], cath[:, 1], catx[:, 0], catx[:, 1]]
    catT = [persist.tile([128, T], F16, tag=f"catT{dt}", name=f"catT{dt}")
            for dt in range(8)]
    for tt in range(4):
        for dt in range(8):
            tp = ps_sml.tile([128, 128], F16, tag="tp", name=f"tp_cat{tt}_{dt}")
            nc.tensor.transpose(tp, cat16[tt][:, dt * 128:(dt + 1) * 128],
                                ident_h)
            nc.vector.tensor_copy(catT[dt][:, tt * 128:(tt + 1) * 128], tp)

    r0 = const.tile([2, DIM], F32, tag="r0", name="r0_sb")
    nc.gpsimd.memset(r0, 0.0)
    nc.sync.dma_start(out=r0[0:1, :], in_=r_d[0:1, :])

    NV2 = NVALID + 1  # rsubT/rwsT allocation width (col 257 unused)
    rsubT = [persist.tile([128, NV2], F16, tag=f"rsubT{dt}", name=f"rsubT{dt}")
             for dt in range(8)]
    for rt in range(2):
        for dt in range(8):
            tp = ps_sml.tile([128, 128], F16, tag="tp", name=f"tp_r{rt}_{dt}")
            nc.tensor.transpose(tp, r16[:, rt, dt * 128:(dt + 1) * 128],
                                ident_h)
            nc.scalar.copy(rsubT[dt][:, rt * 128:(rt + 1) * 128], tp)
    for dt in range(8):
        tp = ps_sml.tile([128, 2], F32, tag="tp", name=f"tp_r0_{dt}")
        nc.tensor.transpose(tp, r0[:, dt * 128:(dt + 1) * 128], ident[0:2, 0:2])
        nc.scalar.copy(rsubT[dt][:, 256:258], tp)

    # ---------------- projections ----------------
    # q_T (x tokens only) -> qu_T, qv_T [128 feat, 256 tok]
    quT = [persist.tile([128, N], F16, tag=f"quT{ft}", name=f"quT{ft}")
           for ft in range(8)]
    qvT = [persist.tile([128, N], F16, tag=f"qvT{ft}", name=f"qvT{ft}")
           for ft in range(8)]
    for ft in range(8):
        pq = ps_mid.tile([128, N], F32, tag="mid", name=f"ps_q{ft}")
        for dt in range(8):
            nc.tensor.matmul(pq, wsl(wq_t, dt)[:, ft * 128:(ft + 1) * 128],
                             catT[dt][:, M:T], start=(dt == 0), stop=(dt == 7))
        nc.vector.tensor_scalar_add(quT[ft], pq, uu)
        nc.vector.tensor_scalar_add(qvT[ft], pq, vv)

    # RWs_T[ft] = [128 feat, 257 offsets] (col 257 unused)
    rwsT = [persist.tile([128, NV2], F16, tag=f"rwsT{ft}", name=f"rwsT{ft}")
            for ft in range(8)]
    for ft in range(8):
        pr = ps_mid.tile([128, NV2], F32, tag="mid", name=f"ps_rw{ft}")
        for dt in range(8):
            nc.tensor.matmul(pr, wsl(wkr_t, dt)[:, ft * 128:(ft + 1) * 128],
                             rsubT[dt], start=(dt == 0), stop=(dt == 7))
        nc.scalar.copy(rwsT[ft], pr)

    # k loads now; k_T[ft] = [128 feat, 512 tok]
    for t_, src in wk_t:
        nc.gpsimd.dma_start(out=t_, in_=src)
    kT = [persist.tile([128, T], F16, tag=f"kT{ft}", name=f"kT{ft}")
          for ft in range(8)]
    for ft in range(8):
        pk = ps_mid.tile([128, T], F32, tag="mid", name=f"ps_k{ft}")
        for dt in range(8):
            nc.tensor.matmul(pk, wsl(wk_t, dt)[:, ft * 128:(ft + 1) * 128],
                             catT[dt], start=(dt == 0), stop=(dt == 7))
        nc.scalar.copy(kT[ft], pk)

    # val[tt] = [128 tok, 1024 feat]; wo loads issued here so they stream
    # during attention instead of trailing the gpsimd queue.
    for t_, src in wv_t + wo_t:
        nc.gpsimd.dma_start(out=t_, in_=src)
    val = [persist.tile([128, INNER], F16, tag=f"val{tt}", name=f"val{tt}")
           for tt in range(4)]
    for tt in range(4):
        pv = [ps_mid.tile([128, 512], F32, tag="mid", name=f"ps_v{tt}_{nh}")
              for nh in range(2)]
        for dt in range(8):
            lhs = catT[dt][:, tt * 128:(tt + 1) * 128]
            for nh in range(2):
                nc.tensor.matmul(pv[nh],
                                 lhs,
                                 wsl(wv_t, dt)[:, nh * 512:(nh + 1) * 512],
                                 start=(dt == 0), stop=(dt == 7))
        for nh in range(2):
            nc.scalar.copy(val[tt][:, nh * 512:(nh + 1) * 512], pv[nh])

    # ---------------- attention (head pairs, software pipelined) -------------
    attn_outT = [persist.tile([128, N], F16, tag=f"aoT{ft}", name=f"aoT{ft}")
                 for ft in range(8)]
    last_read = [None] * NGRP

    def compute_scores(ft):
        """BD + A matmuls, band write DMA, accum-add band read DMA."""
        goff = (ft % NGRP) * 4 * 128 * SW
        bsb = work.tile([128, 4, NVALID], F16, tag="bsb", name=f"bsb{ft}")
        dots = work.tile([128, 4, WIN], F16, tag="dots", name=f"dots{ft}")

        # BD = (q+v) @ RWs^T ; two heads row-tiled, issued back-to-back
        for qb in range(2):
            qsl = slice(qb * 128, (qb + 1) * 128)
            pbs = []
            for hh in range(2):
                pb = ps_mid.tile([128, NVALID], F32, tag="mid",
                                 name=f"pb{ft}_{qb}_{hh}")
                nc.tensor.matmul(pb, qvT[ft][hh * 64:(hh + 1) * 64, qsl],
                                 rwsT[ft][hh * 64:(hh + 1) * 64, 0:NVALID],
                                 start=True, stop=True)
                pbs.append(pb)
            for hh in range(2):
                kk = hh * 2 + qb
                if kk % 2 == 0:
                    nc.vector.tensor_copy(bsb[:, kk], pbs[hh])
                else:
                    nc.scalar.copy(bsb[:, kk], pbs[hh])

        w_inst = nc.sync.dma_start(
            out=bass.AP(bds_d, goff + VAL0,
                        [[SW, 128], [128 * SW, 4], [1, NVALID]]),
            in_=bsb)
        grp = ft % NGRP
        if last_read[grp] is not None:
            add_dep_helper(w_inst.ins, last_read[grp].ins, sync=True,
                           reason="scratch WAR reuse")

        # A = (q+u) @ k^T over the live 384-key window; row-tiled head pair
        for qb in range(2):
            qsl = slice(qb * 128, (qb + 1) * 128)
            pas = []
            for hh in range(2):
                pa = ps_mid.tile([128, WIN], F32, tag="mid",
                                 name=f"pa{ft}_{qb}_{hh}")
                nc.tensor.matmul(pa, quT[ft][hh * 64:(hh + 1) * 64, qsl],
                                 kT[ft][hh * 64:(hh + 1) * 64,
                                        qb * 128:qb * 128 + WIN],
                                 start=True, stop=True)
                pas.append(pa)
            for hh in range(2):
                kk = hh * 2 + qb
                if kk % 2 == 0:
                    nc.vector.tensor_copy(dots[:, kk], pas[hh])
                else:
                    nc.scalar.copy(dots[:, kk], pas[hh])

        # band+mask: batched shear read, then added onto term_a in-place
        band = work.tile([128, 4, WIN], F16, tag="band", name=f"band{ft}")
        r_inst = nc.scalar.dma_start(
            out=band,
            in_=bass.AP(bds_d, goff + VAL0,
                        [[SW - 1, 128], [128 * SW, 4], [1, WIN]]))
        add_dep_helper(r_inst.ins, w_inst.ins, sync=True,
                       reason="band RAW on scratch")
        for zi in zinit:
            add_dep_helper(r_inst.ins, zi.ins, sync=True,
                           reason="band RAW on mask-init")
        last_read[grp] = r_inst
        for kk in range(4):
            nc.vector.tensor_add(dots[:, kk], dots[:, kk], band[:, kk])
        return dots

    def consume(ft, dots):
        """exp, row sums, normalize, transpose to key-major, AV matmuls."""
        expt = work.tile([128, 4, WIN], F16, tag="expt", name=f"expt{ft}")
        exptn = work.tile([128, 4, WIN], F16, tag="exptn", name=f"exptn{ft}")
        ssum = work.tile([128, 4], F32, tag="ssum", name=f"ssum{ft}")
        rcp = work.tile([128, 4], F32, tag="rcp", name=f"rcp{ft}")

        nc.scalar.activation(expt, dots, AF.Exp, bias=0.0, scale=SCALE)
        nc.vector.tensor_reduce(ssum, expt, axis=AX_X, op=ALU_ADD)
        nc.vector.reciprocal(rcp, ssum)
        for kk in range(4):
            eng = nc.vector if kk < 2 else nc.gpsimd
            eng.tensor_scalar_mul(exptn[:, kk], expt[:, kk],
                                  rcp[:, kk:kk + 1])

        attnT = [[work.tile([128, N], F16, tag=f"attnT{hh}_{jt}",
                            name=f"attnT{ft}_{hh}_{jt}")
                  for jt in range(4)] for hh in range(2)]
        for hh in range(2):
            nc.gpsimd.memset(attnT[hh][0][:, 128:256], 0.0)
            nc.gpsimd.memset(attnT[hh][3][:, 0:128], 0.0)

        nt = 0
        for kk in range(4):
            hh, qb = kk // 2, kk % 2
            qsl = slice(qb * 128, (qb + 1) * 128)
            for w in range(3):
                jt = qb + w
                tp = ps_sml.tile([128, 128], F16, tag="tp",
                                 name=f"tp_e{ft}_{kk}_{w}")
                nc.tensor.transpose(tp, exptn[:, kk, w * 128:(w + 1) * 128],
                                    ident_h)
                if nt % 2 == 0:
                    nc.vector.tensor_copy(attnT[hh][jt][:, qsl], tp)
                else:
                    nc.scalar.copy(attnT[hh][jt][:, qsl], tp)
                nt += 1

        # AV: one accumulation chain per head (separate PSUM banks)
        for hh in range(2):
            pav = ps_sml.tile([64, N], F32, tag="tp", name=f"ps_av{ft}_{hh}")
            for jt in range(4):
                nc.tensor.matmul(pav,
                                 val[jt][:, (2 * ft + hh) * 64:
                                         (2 * ft + hh + 1) * 64],
                                 attnT[hh][jt],
                                 start=(jt == 0), stop=(jt == 3))
            nc.scalar.copy(attn_outT[ft][hh * 64:(hh + 1) * 64, :], pav)

    prev = None
    for ft in range(8):
        d = compute_scores(ft)
        if prev is not None:
            consume(ft - 1, prev)
        prev = d
    consume(7, prev)

    # ---------------- output projection ----------------
    for tt in range(2):
        pp = [ps_mid.tile([128, 512], F32, tag="mid", name=f"ps_o{tt}_{nh}")
              for nh in range(2)]
        for itile in range(8):
            lhs = attn_outT[itile][:, tt * 128:(tt + 1) * 128]
            for nh in range(2):
                nc.tensor.matmul(pp[nh],
                                 lhs,
                                 wsl(wo_t, itile)[:, nh * 512:(nh + 1) * 512],
                                 start=(itile == 0), stop=(itile == 7))
        osb = work.tile([128, DIM], F32, tag="osb", name=f"osb{tt}")
        for nh in range(2):
            nc.scalar.copy(osb[:, nh * 512:(nh + 1) * 512], pp[nh])
        nc.sync.dma_start(out=out_d[tt * 128:(tt + 1) * 128, :], in_=osb)


_NC_CACHE = {}


def _get_nc():
    if "nc" not in _NC_CACHE:
        _NC_CACHE["nc"] = build_kernel()
    return _NC_CACHE["nc"]


def _run(inputs, trace=False):
    x = np.ascontiguousarray(np.asarray(inputs["x"], dtype=np.float32))
    h = np.ascontiguousarray(np.asarray(inputs["h"], dtype=np.float32))
    wqkv = np.ascontiguousarray(np.asarray(inputs["Wqkv"], dtype=np.float32))
    wkr = np.ascontiguousarray(np.asarray(inputs["Wkr"], dtype=np.float32))
    r = np.ascontiguousarray(np.asarray(inputs["R"], dtype=np.float32))
    u = np.asarray(inputs["u"], dtype=np.float32)
    v = np.asarray(inputs["v"], dtype=np.float32)
    wout = np.ascontiguousarray(np.asarray(inputs["Wout"], dtype=np.float32))
    uu = np.ascontiguousarray(np.tile(u, 2).reshape(128, 1))
    vv = np.ascontiguousarray(np.tile(v, 2).reshape(128, 1))

    nc = _get_nc()
    in_maps = [
        {"x": x[b], "h": h[b], "Wqkv": wqkv, "Wkr": wkr, "R": r,
         "uu": uu, "vv": vv, "Wout": wout}
        for b in range(B)
    ]
    res = bass_utils.run_bass_kernel_spmd(
        nc, in_maps, core_ids=list(range(B)), trace=trace)
    out = np.stack([res.results[b]["out"] for b in range(B)])
    return out.astype(np.float32), res


def kernel(**inputs):
    out, _ = _run(inputs, trace=False)
    return out
